# revision 17
# baseline (speedup 1.0000x reference)
"""Trainium2 Bass kernel for nn_AttentionRetrievalHead.

Strategy (8 NeuronCores, memory bank sharded along N):

Phase A (device, per core): stream this core's transposed bf16 memory shard
through the PE computing selection scores avg[b,n] = memory[n] . c[b] where
c[b] = (scale/H) * Wk.T q[b] (the head-mean of per-head scores collapses to a
single dot product, so selection needs one [N,256]@[256,64] matmul).  The bf16
scores are dumped to DRAM, per-superchunk(64) maxes are reduced on the fly,
and the top-64 superchunks per batch row within each quarter-shard are
extracted on-device with the DVE max8/max_index/match_replace instructions
(an f32 iota dither makes row values unique so index recovery never drops
tied chunks; quarter-splitting overlaps extraction with the scan).

Phase B (host glue): merge the cores' selected superchunks, take the top-64
candidate elements per batch by dumped bf16 score, exactly rescore a safety
window around the rank-64 boundary in float64, and pick the final top-64
indices (sorted, ties to lower index, matching jax.lax.top_k).

Phase C (device, batch-sharded 8 per core): gather the selected 64 rows per
batch (host gather = the "all-gather of selected V rows"), and on-device
compute per-head scores, softmax, V projection, context and output
projection, batched 4 batches per instruction where possible.  bk drops out
(softmax shift invariance); bv folds into bo' = Wo@bv + bo.
"""

import hashlib
import os
import shutil
import sys
import tempfile
from contextlib import ExitStack

sys.path.insert(0, "/opt/trn_rl_repo")

import ml_dtypes
import numpy as np

import concourse.bass as bass
import concourse.bacc as bacc
import concourse.bass2jax as bass2jax
import concourse.mybir as mybir
import concourse.tile as tile
from concourse.bass_utils import compile_bir_kernel as _orig_compile_bir_kernel
from concourse.bass_utils import run_bass_kernel_spmd

DT = mybir.dt

B = 64
DM = 256
H = 8
HD = 32
N = 262144
NCORES = 8
N_CORE = N // NCORES  # 32768
NT = 512  # matmul subtile width
NT_LD = 4096  # load/dump tile width
N_LD = N_CORE // NT_LD  # 8
SUB = NT_LD // NT  # 8
SC = 64  # superchunk size
N_SC = N_CORE // SC  # 512
NQ = 4  # extraction quarters
Q_SC = N_SC // NQ  # 128 superchunks per quarter
K_SEL = 64  # superchunks selected per batch row per quarter
ROUNDS = K_SEL // 8
DITHER = 2e-6
NEG = -1.0e30
BC = B // NCORES  # 8 batches per core in phase C
GB = 4  # phase C batch-group size
K = 64
SCALE = HD ** -0.5
RESCORE_WINDOW = 0.05

# ---------------------------------------------------------------------------
# NEFF disk cache: walrus compiles take minutes; key on the BIR json hash so
# repeated runs of the same program skip the compiler entirely.
_NEFF_CACHE_DIR = os.path.expanduser("~/.cache/bass_neff_cache")


def _cached_compile_bir_kernel(bir_json, tmpdir, neff_name="file.neff"):
    data = bir_json if isinstance(bir_json, bytes) else bir_json.encode()
    h = hashlib.sha256(data).hexdigest()
    cpath = os.path.join(_NEFF_CACHE_DIR, h + ".neff")
    dst = os.path.join(tmpdir, neff_name)
    if os.path.exists(cpath):
        shutil.copyfile(cpath, dst)
        return dst
    out = _orig_compile_bir_kernel(bir_json, tmpdir, neff_name)
    try:
        os.makedirs(_NEFF_CACHE_DIR, exist_ok=True)
        fd, tmp = tempfile.mkstemp(dir=_NEFF_CACHE_DIR)
        os.close(fd)
        shutil.copyfile(out, tmp)
        os.replace(tmp, cpath)
    except OSError:
        pass
    return out


bass2jax.compile_bir_kernel = _cached_compile_bir_kernel


# ---------------------------------------------------------------------------
# Phase A program


def _emit_extract_quarter(nc, ex_pool, m_sb, dith, ids_sb, q):
    """Extract top-64 superchunks (of this quarter's 128) per batch row."""
    mq = ex_pool.tile([B, Q_SC], DT.float32, tag=f"mq{q}")
    nc.vector.tensor_tensor(
        mq[:], m_sb[:, bass.ts(q, Q_SC)], dith[:], op=mybir.AluOpType.add
    )
    v8 = ex_pool.tile([B, 8], DT.float32, tag=f"v8{q}")
    for r in range(ROUNDS):
        nc.vector.max(v8[:], mq[:])
        nc.vector.max_index(ids_sb[:, q, bass.ts(r, 8)], v8[:], mq[:])
        nc.vector.match_replace(mq[:], v8[:], mq[:], NEG)


def _build_phase_a(has_bias: bool):
    nc = bacc.Bacc("TRN2", target_bir_lowering=False, debug=False)
    memT = nc.dram_tensor(
        "memT", [2, 128, N_CORE], DT.bfloat16, kind="ExternalInput"
    ).ap()
    qk = nc.dram_tensor("qk", [2, 128, B], DT.bfloat16, kind="ExternalInput").ap()
    if has_bias:
        bias_d = nc.dram_tensor(
            "bias", [1, N_CORE], DT.bfloat16, kind="ExternalInput"
        ).ap()
    sdump = nc.dram_tensor("sdump", [B, N_CORE], DT.bfloat16, kind="ExternalOutput").ap()
    ids_d = nc.dram_tensor("ids", [B, NQ, K_SEL], DT.uint16, kind="ExternalOutput").ap()

    with tile.TileContext(nc) as tc, ExitStack() as ctx:
        const_pool = ctx.enter_context(tc.tile_pool(name="const", bufs=1))
        mem_pool = ctx.enter_context(tc.tile_pool(name="mem", bufs=3))
        sc_pool = ctx.enter_context(tc.tile_pool(name="sc", bufs=3))
        ps_pool = ctx.enter_context(tc.tile_pool(name="ps", bufs=4, space="PSUM"))
        ex_pool = ctx.enter_context(tc.tile_pool(name="ex", bufs=1))

        qk_sb = const_pool.tile([128, 2, B], DT.bfloat16)
        for c in range(2):
            nc.sync.dma_start(qk_sb[:, c, :], qk[c])
        if has_bias:
            bias_sb = const_pool.tile([1, N_CORE], DT.bfloat16)
            nc.sync.dma_start(bias_sb[:], bias_d[:])
            ones_sb = const_pool.tile([1, B], DT.bfloat16)
            nc.vector.memset(ones_sb[:], 1.0)

        # superchunk maxes (bf16 maxes of bf16 scores are exact)
        m_sb = ex_pool.tile([B, N_SC], DT.bfloat16)
        ids_sb = ex_pool.tile([B, NQ, K_SEL], DT.uint16)

        # dither ramp: makes all values in a quarter-row distinct
        iota_i = ex_pool.tile([B, Q_SC], DT.int32)
        nc.gpsimd.iota(iota_i[:], pattern=[[1, Q_SC]], base=0, channel_multiplier=0)
        dith = ex_pool.tile([B, Q_SC], DT.float32)
        nc.vector.tensor_copy(dith[:], iota_i[:])
        nc.vector.tensor_scalar(dith[:], dith[:], DITHER, None, op0=mybir.AluOpType.mult)

        tiles_per_q = N_SC // (NT_LD // SC) // NQ  # load tiles per quarter = 2

        for i in range(N_LD):
            mt = mem_pool.tile([128, 2, NT_LD], DT.bfloat16, tag="mt")
            for c in range(2):
                nc.sync.dma_start(mt[:, c, :], memT[c, :, bass.ts(i, NT_LD)])
            sb = sc_pool.tile([B, NT_LD], DT.bfloat16, tag="sb")
            for j in range(SUB):
                ps = ps_pool.tile([B, NT], DT.float32)
                nc.tensor.matmul(
                    ps[:],
                    qk_sb[:, 0, :],
                    mt[:, 0, bass.ts(j, NT)],
                    start=True,
                    stop=False,
                )
                nc.tensor.matmul(
                    ps[:],
                    qk_sb[:, 1, :],
                    mt[:, 1, bass.ts(j, NT)],
                    start=False,
                    stop=not has_bias,
                )
                if has_bias:
                    nc.tensor.matmul(
                        ps[:],
                        ones_sb[:],
                        bias_sb[:, i * NT_LD + j * NT :][:, :NT],
                        start=False,
                        stop=True,
                    )
                nc.scalar.activation(
                    sb[:, bass.ts(j, NT)], ps[:], mybir.ActivationFunctionType.Copy
                )
                nc.vector.reduce_max(
                    m_sb[:, i * (NT_LD // SC) + j * (NT // SC) :][:, : NT // SC],
                    sb[:, bass.ts(j, NT)].rearrange("b (c e) -> b c e", e=SC),
                    axis=mybir.AxisListType.X,
                )
            nc.sync.dma_start(sdump[:, bass.ts(i, NT_LD)], sb[:])
            # as soon as a quarter's maxes are complete, extract it
            if (i + 1) % tiles_per_q == 0:
                _emit_extract_quarter(
                    nc, ex_pool, m_sb, dith, ids_sb, (i + 1) // tiles_per_q - 1
                )

        nc.sync.dma_start(ids_d[:], ids_sb[:])

    nc.compile()
    return nc


# ---------------------------------------------------------------------------
# Phase C program


def _build_phase_c():
    nc = bacc.Bacc("TRN2", target_bir_lowering=False, debug=False)
    gmT = nc.dram_tensor("gmT", [BC, 2, 128, K], DT.float32, kind="ExternalInput").ap()
    qkm = nc.dram_tensor("qkm", [BC, 2, 128, H], DT.float32, kind="ExternalInput").ap()
    wvT = nc.dram_tensor("wvT", [2, 128, 256], DT.float32, kind="ExternalInput").ap()
    woT = nc.dram_tensor("woT", [2, 128, 2, 128], DT.float32, kind="ExternalInput").ap()
    bo2 = nc.dram_tensor("bo2", [2, 128, 1], DT.float32, kind="ExternalInput").ap()
    ctxT_d = nc.dram_tensor("ctxT", [2, 128, BC], DT.float32, kind="ExternalOutput").ap()
    aw_d = nc.dram_tensor("aw", [K, BC], DT.float32, kind="ExternalOutput").ap()

    ngrp = BC // GB

    with tile.TileContext(nc) as tc, ExitStack() as ctx:
        cpool = ctx.enter_context(tc.tile_pool(name="c", bufs=1))
        wpool = ctx.enter_context(tc.tile_pool(name="w", bufs=2))
        ppool = ctx.enter_context(tc.tile_pool(name="pp", bufs=2, space="PSUM"))
        opool = ctx.enter_context(tc.tile_pool(name="o", bufs=1))

        wv_sb = cpool.tile([128, 2, 256], DT.float32)
        for c in range(2):
            for hc in range(2):
                nc.sync.dma_start(
                    wv_sb[:, c, bass.ts(hc, 128)], wvT[c, :, bass.ts(hc, 128)]
                )
        wo_sb = cpool.tile([128, 2, 2, 128], DT.float32)
        for c in range(2):
            for oc in range(2):
                nc.sync.dma_start(wo_sb[:, c, oc, :], woT[c, :, oc, :])
        bo_sb = cpool.tile([128, 2], DT.float32)
        for c in range(2):
            nc.sync.dma_start(bo_sb[:, c : c + 1], bo2[c])

        ctx_sb = opool.tile([128, 2, BC], DT.float32)
        aw_sb = opool.tile([K, BC], DT.float32)
        ebias = cpool.tile([128, 1], DT.float32)
        nc.vector.memset(ebias[:], -6.0)

        for g in range(ngrp):
            g0 = g * GB
            gm4 = wpool.tile([128, GB, 2, K], DT.float32, tag="gm4")
            for b in range(GB):
                nc.sync.dma_start(
                    gm4[:, b], gmT[g0 + b].rearrange("c m j -> m c j")
                )
            qkm4 = wpool.tile([128, GB, 2, H], DT.float32, tag="qkm4")
            nc.sync.dma_start(
                qkm4[:], qkm[g0 : g0 + GB].rearrange("b mc m h -> m b mc h")
            )
            # replicate QK columns 32x on-device: col p of chunk c <-> head 4c+p//32
            qh4 = wpool.tile([128, GB, 2, 2, 4, 32], DT.float32, tag="qh4")
            for mc in range(2):
                nc.vector.tensor_copy(
                    qh4[:, :, :, mc],
                    qkm4[:, :, mc, :]
                    .rearrange("p b (c hh) -> p b c hh", c=2)
                    .broadcast_to([128, GB, 2, 4, 32]),
                )

            # per-head scores, replicated: srep[c][p, j] = s[4c + p//32, j]
            srep = ppool.tile([128, 2, GB, K], DT.float32, tag="srep")
            for c in range(2):
                for b in range(GB):
                    for mc in range(2):
                        nc.tensor.matmul(
                            srep[:, c, b, :],
                            qh4[:, b, c, mc, :],
                            gm4[:, b, mc, :],
                            start=(mc == 0),
                            stop=(mc == 1),
                        )
            # exp without max-subtraction (scores are O(10), far from
            # overflow; the -6 shift cancels exactly in the normalization)
            att4 = wpool.tile([128, 2, GB, K], DT.float32, tag="att4")
            nc.scalar.activation(
                att4[:], srep[:], mybir.ActivationFunctionType.Exp, bias=ebias[:]
            )
            sm = wpool.tile([128, 2 * GB], DT.float32, tag="sm")
            nc.vector.reduce_sum(
                sm[:],
                att4[:].rearrange("p c b j -> p (c b) j"),
                axis=mybir.AxisListType.X,
            )
            rec = wpool.tile([128, 2, GB], DT.float32, tag="rec")
            nc.vector.reciprocal(rec[:].rearrange("p c b -> p (c b)"), sm[:])
            rsc = wpool.tile([128, 2, GB], DT.float32, tag="rsc")
            nc.vector.tensor_scalar(
                rsc[:], rec[:], 1.0 / (32 * H), None, op0=mybir.AluOpType.mult
            )

            # vT[c][p, j] per batch
            vps = ppool.tile([128, 2, GB, K], DT.float32, tag="vps")
            for c in range(2):
                for b in range(GB):
                    for mc in range(2):
                        nc.tensor.matmul(
                            vps[:, c, b, :],
                            wv_sb[:, mc, bass.ts(c, 128)],
                            gm4[:, b, mc, :],
                            start=(mc == 0),
                            stop=(mc == 1),
                        )

            # ctx_pre[c, b][p] = (sum_j vt * e) * rec
            prod = wpool.tile([128, 2, GB, K], DT.float32, tag="prod")
            nc.vector.tensor_tensor(
                prod[:], vps[:], att4[:], op=mybir.AluOpType.mult
            )
            cpre = wpool.tile([128, 2, GB], DT.float32, tag="cpre")
            nc.vector.reduce_sum(
                cpre[:],
                prod[:].rearrange("p c b j -> p (c b) j"),
                axis=mybir.AxisListType.X,
            )
            nc.vector.tensor_tensor(
                cpre[:], cpre[:], rec[:], op=mybir.AluOpType.mult
            )

            # attention weights via PE partition-sum: aw[j,b] =
            # sum_p e[p,j]*rsc[p] over both chunks (32 copies per head / 256)
            awp = ppool.tile([K, GB], DT.float32, tag="awp")
            for b in range(GB):
                for c in range(2):
                    nc.tensor.matmul(
                        awp[:, b : b + 1],
                        att4[:, c, b, :],
                        rsc[:, c, b : b + 1],
                        start=(c == 0),
                        stop=(c == 1),
                    )
            nc.vector.tensor_copy(aw_sb[:, g0 : g0 + GB], awp[:])

            # output projection + fused bias
            ops4 = ppool.tile([128, 2, GB], DT.float32, tag="ops4")
            for oc in range(2):
                for mc in range(2):
                    nc.tensor.matmul(
                        ops4[:, oc, :],
                        wo_sb[:, mc, oc, :],
                        cpre[:, mc, :],
                        start=(mc == 0),
                        stop=(mc == 1),
                    )
            nc.vector.tensor_tensor(
                ctx_sb[:, :, g0 : g0 + GB],
                ops4[:],
                bo_sb[:].broadcast_to([128, 2, GB]),
                op=mybir.AluOpType.add,
            )

        nc.sync.dma_start(ctxT_d[0], ctx_sb[:, 0, :])
        nc.sync.dma_start(ctxT_d[1], ctx_sb[:, 1, :])
        nc.sync.dma_start(aw_d[:], aw_sb[:])

    nc.compile()
    return nc


_PROGRAMS = {}


def _get_program(name, builder, *args):
    key = (name,) + args
    if key not in _PROGRAMS:
        _PROGRAMS[key] = builder(*args)
    return _PROGRAMS[key]


# ---------------------------------------------------------------------------


def _to_f32(x):
    return np.asarray(x, dtype=np.float32)


def kernel(query, memory, Wk, bk, Wv, bv, Wo, bo, memory_mask, top_k):
    query = _to_f32(query)
    memory = _to_f32(memory)
    Wk = _to_f32(Wk)
    Wv = _to_f32(Wv)
    Wo = _to_f32(Wo)
    bk = _to_f32(bk)
    bv = _to_f32(bv)
    bo = _to_f32(bo)
    mask = np.asarray(memory_mask, dtype=bool)
    k_req = int(top_k)
    assert query.shape == (B, DM) and memory.shape == (N, DM) and k_req == K

    has_bias = not bool(mask.all())

    # ---- host prep for phase A
    # c[b] = (scale/H) * Wk.T @ q[b]; bk shifts every slot equally -> no-op
    Cmat = (Wk.T @ query.T) * (SCALE / H)  # [DM, B] f32
    qk_bf = Cmat.astype(ml_dtypes.bfloat16).reshape(2, 128, B)

    in_maps = []
    for core in range(NCORES):
        shard = memory[core * N_CORE : (core + 1) * N_CORE]  # [N_CORE, DM]
        memT = np.ascontiguousarray(shard.astype(ml_dtypes.bfloat16).T).reshape(
            2, 128, N_CORE
        )
        m = {"memT": memT, "qk": qk_bf}
        if has_bias:
            mslice = mask[core * N_CORE : (core + 1) * N_CORE]
            m["bias"] = np.where(mslice, 0.0, NEG).astype(ml_dtypes.bfloat16).reshape(
                1, N_CORE
            )
        in_maps.append(m)

    nc_a = _get_program("phase_a", _build_phase_a, has_bias)
    res_a = run_bass_kernel_spmd(nc_a, in_maps, core_ids=list(range(NCORES)))

    # ---- phase B: host merge + exact rescore of the boundary window
    eoff = np.arange(SC, dtype=np.int64)
    vals_parts = []
    gidx_parts = []
    for core in range(NCORES):
        sdump = np.asarray(res_a.results[core]["sdump"])  # [B, N_CORE] bf16
        ids = res_a.results[core]["ids"].astype(np.int64)  # [B, NQ, K_SEL]
        sc_glob = ids + (np.arange(NQ) * Q_SC)[None, :, None]  # global superchunk
        off = (sc_glob[..., None] * SC + eoff).reshape(B, NQ * K_SEL * SC)
        vals = np.take_along_axis(sdump, off, axis=1).astype(np.float32)
        vals_parts.append(vals)
        gidx_parts.append(off + core * N_CORE)
    cand_vals = np.concatenate(vals_parts, axis=1)
    cand_idx = np.concatenate(gidx_parts, axis=1)

    v64 = np.partition(cand_vals, -K, axis=1)[:, -K]  # 64th largest bf16 value
    Cmat64 = (Wk.T.astype(np.float64) @ query.T.astype(np.float64)) * (SCALE / H)

    top_idx = np.empty((B, K), dtype=np.int32)
    for b in range(B):
        keep = cand_vals[b] >= (v64[b] - RESCORE_WINDOW)
        idxs = np.unique(cand_idx[b][keep])
        exact = memory[idxs].astype(np.float64) @ Cmat64[:, b]
        if has_bias:
            exact = np.where(mask[idxs], exact, -np.inf)
        order = np.lexsort((idxs, -exact))[:K]
        top_idx[b] = idxs[order].astype(np.int32)

    # ---- phase C
    rows = memory[top_idx.reshape(-1)].reshape(B, K, DM)
    gmT_all = np.ascontiguousarray(rows.transpose(0, 2, 1)).reshape(B, 2, 128, K)
    QK = (
        np.einsum(
            "hdm,bhd->bmh",
            Wk.reshape(H, HD, DM),
            query.reshape(B, H, HD),
            optimize=True,
        )
        * SCALE
    ).astype(np.float32)
    qkm_all = QK.reshape(B, 2, 128, H)

    wvT = np.ascontiguousarray(Wv.T).reshape(2, 128, 256)
    woT = np.ascontiguousarray(Wo.T).reshape(2, 128, 2, 128)
    bo2 = (Wo @ bv + bo).astype(np.float32).reshape(2, 128, 1)

    in_maps_c = []
    for core in range(NCORES):
        sl = slice(core * BC, (core + 1) * BC)
        in_maps_c.append(
            {
                "gmT": gmT_all[sl],
                "qkm": qkm_all[sl],
                "wvT": wvT,
                "woT": woT,
                "bo2": bo2,
            }
        )

    nc_c = _get_program("phase_c", _build_phase_c)
    res_c = run_bass_kernel_spmd(nc_c, in_maps_c, core_ids=list(range(NCORES)))

    context = np.empty((B, DM), np.float32)
    attn_weights = np.empty((B, K), np.float32)
    for core in range(NCORES):
        ctxT = np.asarray(res_c.results[core]["ctxT"]).reshape(DM, BC)
        aw = np.asarray(res_c.results[core]["aw"])  # [K, BC]
        context[core * BC : (core + 1) * BC] = ctxT.T
        attn_weights[core * BC : (core + 1) * BC] = aw.T

    return context, attn_weights, top_idx
